# revision 27
# baseline (speedup 1.0000x reference)
"""DCNv2 (modulated deformable conv 3x3 + BatchNorm + SiLU) on Trainium2.

Full problem: x[4,256,80,80]; offset/mask conv (256->27); bilinear-sampled
modulated deformable conv (256->256); BN (batch stats); SiLU.

SPMD over 8 NeuronCores: shard = (batch, row-half) = 1 batch x 40 rows.

v2: software-pipelined superblocks so the Pool engine (gpsimd local_scatter,
the per-core floor) is never starved:
  - per superblock k: om conv (PE) -> transpose (PE) -> coord math (DVE)
    -> repartition DMAs are emitted BEFORE the scatter+matmul phase of
    superblock k-1, so per-engine in-order queues overlap across blocks.
  - PSUM->SBUF copies ride the Activation engine (ACTF.Copy), BN partial
    stats ride activation accum_out; DVE only does coordinate math.
  - x window is a rolling 12-slot buffer fed one row/iteration (no
    redundant window DMA).
  - y goes to a DRAM scratch per GEMM flush and is re-read in the tail
    (saves 25KB SBUF).

Per-row sampling math (unchanged from v1):
  GPSIMD local_scatter builds the selection matrix
     S_g[w'(part), drow-local, tloc, p]  (7*3*80 = 1680 slots, auto-zeroed)
  and the tensor engine computes gather+bilinear+mask as
     val[c, (t,p)] += sum_{drow} xwin[row, w', c] * S_g[w', ...]
  accumulating 7 window rows in PSUM; every RB rows the main GEMM
     y[o,p] += W[o,(c,t)] val[(c,t),p]  accumulates in PSUM.
All matmuls bf16 inputs with fp32 PSUM accumulation.
"""

import dataclasses
import numpy as np

import concourse.bacc as bacc
import concourse.bass as bass
import concourse.tile as tile
from concourse import mybir
from concourse.masks import make_identity

F32 = mybir.dt.float32
BF16 = mybir.dt.bfloat16
I32 = mybir.dt.int32
I16 = mybir.dt.int16
ALU = mybir.AluOpType
ACTF = mybir.ActivationFunctionType
BF16_NP = mybir.dt.np(BF16)
AX = mybir.AxisListType


@dataclasses.dataclass(frozen=True)
class Cfg:
    C: int = 256
    CO: int = 256
    H: int = 80
    W: int = 80
    NR: int = 40            # output rows per shard
    B_total: int = 4
    n_cores: int = 8
    M: int = 4              # sampling margin rows: floor(off) in [-M+1, M-2]
    RB: int = 4             # rows per main-GEMM block
    WINB: int = 12          # rolling x-window slots
    eps: float = 1e-5
    use_collective: bool = True
    use_silu: bool = True

    T: int = 9
    KY: int = 3             # tap-row groups
    TL: int = 3             # taps per group

    @property
    def CC(self): return self.C // 128

    @property
    def OC(self): return self.CO // 128

    @property
    def WIN(self): return 2 * self.M + 1          # window rows (9)

    @property
    def GW(self): return 2 * self.M - 1           # per-group window rows (7)

    @property
    def ND(self): return 2 * self.M + 1           # column shifts d' (9)

    @property
    def NTP(self): return self.T * self.W         # val cols, tap-major

    @property
    def GSLOT(self): return self.GW * self.TL * self.W   # local_scatter dst (1680)

    @property
    def NCAND(self): return self.ND * self.TL * 4        # candidates/group (108)

    @property
    def PMROWS(self): return self.NR + 2 * self.M        # 48

    @property
    def SBS(self):  # superblock row counts (sum == NR)
        return (8, 10, 11, 11)

    @property
    def SBMAX(self): return max(self.SBS)

    @property
    def bn_count(self): return float(self.B_total * self.H * self.W)


CFG = Cfg()


def _row_tiles(nr, maxrows):
    out, j = [], 0
    while j < nr:
        out.append((j, min(maxrows, nr - j)))
        j += maxrows
    return out


def build_nc(cfg: Cfg = CFG, debug: bool = False):
    nc = bacc.Bacc("TRN2", target_bir_lowering=False,
                   num_devices=cfg.n_cores if cfg.use_collective else None)
    C, CO, H, W, NR, T, M = cfg.C, cfg.CO, cfg.H, cfg.W, cfg.NR, cfg.T, cfg.M
    CC, OC, WIN, GW, ND = cfg.CC, cfg.OC, cfg.WIN, cfg.GW, cfg.ND
    KY, TL, NTP, RB, WINB = cfg.KY, cfg.TL, cfg.NTP, cfg.RB, cfg.WINB
    SBS, SBM = cfg.SBS, cfg.SBMAX
    NP = NR * W
    XW = W + 2
    GSL = TL * W
    P0 = 0
    nflush = NR // RB  # 10
    nsp = 2
    wsp = W // nsp     # 40

    # ---------------- I/O ----------------
    x_cm = nc.dram_tensor("x_cm", [128, CC * (NR + 2) * XW], BF16, kind="ExternalInput")
    x_pm = nc.dram_tensor("x_pm", [cfg.PMROWS * 128, C], BF16, kind="ExternalInput")
    w_om_l = nc.dram_tensor("w_om_l", [128, T * CC * 32], BF16, kind="ExternalInput")
    b_om_t = nc.dram_tensor("b_om_t", [32, 1], F32, kind="ExternalInput")
    w_ct_t = nc.dram_tensor("w_ct_t", [128, T * CC * CO], BF16, kind="ExternalInput")
    coef_t = nc.dram_tensor("coef_t", [W, 5 * NR * T], F32, kind="ExternalInput")
    gb_t = nc.dram_tensor("gb_t", [128, 2 * OC], F32, kind="ExternalInput")

    y_out = nc.dram_tensor("y_out", [128, OC * NP], F32, kind="ExternalOutput")
    y_scr = nc.dram_tensor("y_scr", [128, OC * NP], BF16)

    if cfg.use_collective:
        cc_in = nc.dram_tensor("cc_in", [128, 2 * OC], F32)
        cc_out = nc.dram_tensor("cc_out", [128, 2 * OC], F32, addr_space="Shared")

    with tile.TileContext(nc) as tc:
        with (
            tc.tile_pool(name="const", bufs=1) as cp,
            tc.tile_pool(name="psa", bufs=6, space="PSUM") as ps_a,
            tc.tile_pool(name="psb", bufs=2, space="PSUM") as ps_b,
        ):
            sp_cm = tc.tile_pool(name="sbp", bufs=2)
            sp = sp_cm.__enter__()
            wp_cm = tc.tile_pool(name="wp", bufs=2)
            wp = wp_cm.__enter__()
            ssp_cm = tc.tile_pool(name="ssp", bufs=6)
            ssp = ssp_cm.__enter__()
            # ---------------- constants ----------------
            # xc first chunk covers SB0+SB1 row tiles; rest loaded late.
            XCHUNK = 16
            xc = cp.tile([128, CC, (NR + 2), XW], BF16)
            xcmv = x_cm[:, :].rearrange("p (c n q) -> p c n q", c=CC, q=XW)
            nc.sync.dma_start(xc[:, :, 0:XCHUNK, :], xcmv[:, :, 0:XCHUNK, :])
            woml = cp.tile([128, T, CC, 32], BF16)
            nc.sync.dma_start(woml[:, :, :, :],
                              w_om_l[:, :].rearrange("p (t c o) -> p t c o", t=T, c=CC))
            bom = cp.tile([32, 1], F32)
            nc.sync.dma_start(bom[:, :], b_om_t[:, :])
            wct = cp.tile([128, T * CC, CO], BF16)

            coef = cp.tile([128, 5, NR, T], F32)
            nc.sync.dma_start(coef[P0:P0 + W, :, :, :],
                              coef_t[:, :].rearrange("p (k r t) -> p k r t",
                                                     k=5, t=T))

            gb = cp.tile([128, 2 * OC], F32)
            ident = cp.tile([128, 128], F32)
            make_identity(nc, ident[:, :])

            omt = cp.tile([128, NR, 27], F32)
            val_sb = cp.tile([128, CC, RB, NTP], BF16)
            # BN partial stats: [kind(sum,sumsq), oi, flush*s]
            parts = cp.tile([128, 2, OC, nflush * nsp], F32)
            stats = cp.tile([128, 2 * OC], F32)

            # rolling x window
            win = cp.tile([128, WINB, C], BF16)
            xpm_v = x_pm[:, :].rearrange("(g p) c -> g p c", p=128)

            def load_win(g, n=1):
                nc.scalar.dma_start(
                    win[:, g % WINB:g % WINB + n, :],
                    xpm_v[g:g + n, :, :].rearrange("g p c -> p g c"))

            for g in range(0, WIN + 1, 2):
                load_win(g, 2)

            sl = slice(P0, P0 + W)

            # ---------- per-superblock producer phases (B, C, D, R) ----------
            def emit_B_tile(k, om_sb, jl, nrt):
                s0 = sum(SBS[:k])
                j0 = s0 + jl
                pt = ps_b.tile([32, 6 * W], F32, tag="mm", name="pt")
                n = nrt * W
                out_ap = pt[:27, 0:n].rearrange("p (r w) -> p r w", w=W)
                first = True
                for t in range(T):
                    ky, kx = t // 3, t % 3
                    for ci in range(CC):
                        rhs = xc[:, ci, j0 + ky:j0 + ky + nrt, kx:kx + W]
                        nc.tensor.matmul(
                            out_ap, lhsT=woml[:, t, ci, 0:27], rhs=rhs,
                            start=first, stop=(t == T - 1 and ci == CC - 1))
                        first = False
                nc.scalar.activation(om_sb[0:27, jl * W:jl * W + n], pt[:27, 0:n],
                                     ACTF.Identity, bias=bom[0:27, :])

            def emit_B(k):
                """om conv rows [s0, s0+sn) -> om_sb (27 partitions)."""
                sn = SBS[k]
                om_sb = sp.tile([32, SBM * W], F32, tag="om_sb", name="om_sb")
                for (jl, nrt) in _row_tiles(sn, 6):
                    emit_B_tile(k, om_sb, jl, nrt)
                return om_sb

            def bc_fillers(k):
                """B-tile + C closures for superblock k (PE work spread)."""
                state = {}

                def b_tile(jl, nrt, first):
                    def f():
                        if first:
                            state["om"] = sp.tile([32, SBM * W], F32,
                                                  tag="om_sb", name="om_sb")
                        emit_B_tile(k, state["om"], jl, nrt)
                    return f

                fillers = [b_tile(jl, nrt, i == 0)
                           for i, (jl, nrt) in enumerate(_row_tiles(SBS[k], 6))]
                fillers.append(lambda: emit_C(k, state["om"]))
                return fillers

            def dr_filler(k, holder):
                def dr():
                    w4p, idxp = emit_D(k)
                    holder["dsb"] = emit_R(k, w4p, idxp)
                return [dr]

            def emit_C(k, om_sb):
                """transpose om -> omt (pixel-on-partition)."""
                s0 = sum(SBS[:k])
                sn = SBS[k]
                for r in range(sn):
                    ptt = ps_a.tile([128, 32], F32, tag="sel")
                    nc.tensor.transpose(ptt[P0:P0 + W, 0:27],
                                        om_sb[0:27, r * W:(r + 1) * W],
                                        ident[0:27, 0:27])
                    nc.scalar.activation(omt[sl, s0 + r, :], ptt[P0:P0 + W, 0:27],
                                         ACTF.Copy)

            def emit_D(k):
                """coordinate math -> w4p (weights) + idxp (scatter indices)."""
                s0 = sum(SBS[:k])
                sn = SBS[k]
                shp = [128, SBM, T]
                rs = slice(s0, s0 + sn)
                tiles = {}

                def mk(tag):
                    t_ = sp.tile(shp, F32, tag=tag, name=tag)
                    tiles[tag] = t_
                    return t_[sl, 0:sn, :]

                def tt(dst, a, b, op):
                    nc.vector.tensor_tensor(dst, a, b, op=op)

                def tsc(dst, a, s1, s2, op0, op1=None):
                    if op1 is None:
                        nc.vector.tensor_scalar(dst, a, s1, None, op0=op0)
                    else:
                        nc.vector.tensor_scalar(dst, a, s1, s2, op0=op0, op1=op1)

                dy = omt[sl, rs, 0:2 * T:2]
                dx = omt[sl, rs, 1:2 * T:2]
                mlog = omt[sl, rs, 2 * T:3 * T]
                yb = coef[sl, 0, rs, :]
                xb = coef[sl, 1, rs, :]
                rj = coef[sl, 2, rs, :]
                ibc = coef[sl, 3, rs, :]
                pb = coef[sl, 4, rs, :]

                ys, xs = mk("ys"), mk("xs")
                tt(ys, dy, yb, ALU.add)
                tt(xs, dx, xb, ALU.add)
                ti = sp.tile(shp, I32, tag="ti", name="ti")[sl, 0:sn, :]
                tf, g = mk("tf"), mk("g")

                def floor_(dst, src):
                    nc.vector.tensor_copy(ti, src)
                    nc.vector.tensor_copy(tf, ti)
                    tt(g, tf, src, ALU.is_gt)
                    tt(dst, tf, g, ALU.subtract)

                y0, x0 = mk("y0"), mk("x0")
                floor_(y0, ys)
                floor_(x0, xs)
                ay, ax_ = mk("ay"), mk("ax")
                tt(ay, ys, y0, ALU.subtract)
                tt(ax_, xs, x0, ALU.subtract)

                msk = mk("msk")
                nc.scalar.activation(msk, mlog, ACTF.Sigmoid)

                def valid(dst, src, lo_src, hi):
                    # dst = (lo_src >= lo) & (lo_src <= hi) evaluated on src+off
                    tsc(g, src, 0.0, None, ALU.is_ge)
                    tsc(dst, src, float(hi), None, ALU.is_le)
                    tt(dst, dst, g, ALU.mult)

                vy0, vy1, vx0, vx1 = mk("vy0"), mk("vy1"), mk("vx0"), mk("vx1")
                y1f, x1f = mk("y1f"), mk("x1f")
                tsc(y1f, y0, 1.0, None, ALU.add)
                tsc(x1f, x0, 1.0, None, ALU.add)
                valid(vy0, y0, y0, H - 1)
                valid(vy1, y1f, y1f, H - 1)
                valid(vx0, x0, x0, W - 1)
                valid(vx1, x1f, x1f, W - 1)

                def clamp(dst, src, hi):
                    tsc(dst, src, 0.0, float(hi), ALU.max, ALU.min)

                y0c, y1c, x0c, x1c = mk("y0c"), mk("y1c"), mk("x0c"), mk("x1c")
                clamp(y0c, y0, H - 1)
                clamp(y1c, y1f, H - 1)
                clamp(x0c, x0, W - 1)
                clamp(x1c, x1f, W - 1)

                a0, a1, b0, b1 = mk("a0"), mk("a1"), mk("b0"), mk("b1")
                tsc(a0, ay, -1.0, 1.0, ALU.mult, ALU.add)
                tt(a0, a0, vy0, ALU.mult)
                tt(a1, ay, vy1, ALU.mult)
                tsc(b0, ax_, -1.0, 1.0, ALU.mult, ALU.add)
                tt(b0, b0, vx0, ALU.mult)
                tt(b0, b0, msk, ALU.mult)
                tt(b1, ax_, vx1, ALU.mult)
                tt(b1, b1, msk, ALU.mult)

                # w4p: bilinear weights (mask folded), bf16, pixel domain
                w4p = sp.tile([128, SBM, T, 4], BF16, tag="w4p", name="w4p")
                nc.vector.memset(w4p[:, :, :, :], 0.0)
                # corner order: 0=(y0,x0) 1=(y0,x1) 2=(y1,x0) 3=(y1,x1)
                corners = [(a0, b0, 0), (a0, b1, 1), (a1, b0, 2), (a1, b1, 3)]
                # nz flags reuse the vy/vx tiles (dead after a/b computed)
                nzt = [vy0, vy1, vx0, vx1]
                for (u, v, i) in corners:
                    tt(w4p[sl, 0:sn, :, i], u, v, ALU.mult)
                    tt(nzt[i], u, v, ALU.mult)
                    tsc(nzt[i], nzt[i], 0.0, None, ALU.not_equal)

                # drow/ib: scatter target = drl*GSL + tloc*W + p  (ibc holds
                # -ky*TL*W + tloc*W + p);  ib reuses ys/xs, q reuses a/b tiles.
                ib0, ib1 = ys, xs
                tt(g, y0c, rj, ALU.subtract)
                tsc(g, g, float(M), None, ALU.add)
                tsc(g, g, float(GSL), None, ALU.mult)
                tt(ib0, g, ibc, ALU.add)
                clamp(ib0, ib0, GW * GSL - 1)
                tsc(ib0, ib0, 1.0, None, ALU.add)      # ib0 + 1
                tt(g, y1c, rj, ALU.subtract)
                tsc(g, g, float(M), None, ALU.add)
                tsc(g, g, float(GSL), None, ALU.mult)
                tt(ib1, g, ibc, ALU.add)
                clamp(ib1, ib1, GW * GSL - 1)
                tsc(ib1, ib1, 1.0, None, ALU.add)      # ib1 + 1

                # q_i = (ib+1)*nz_i - 1  (corner row: 0,1 -> ib0; 2,3 -> ib1)
                # where a candidate is invalid (nz=0) this is -1 already.
                q = [a0, a1, b0, b1]
                tt(q[0], ib0, nzt[0], ALU.mult)
                tt(q[1], ib0, nzt[1], ALU.mult)
                tt(q[2], ib1, nzt[2], ALU.mult)
                tt(q[3], ib1, nzt[3], ALU.mult)
                for i in range(4):
                    tsc(q[i], q[i], -1.0, None, ALU.add)
                neg1 = mk("neg1")
                nc.vector.memset(neg1, -1.0)

                # pc: source-shift keys; pc reuses ay/ax
                pc0, pc1 = ay, ax_
                tt(pc0, pb, x0c, ALU.subtract)
                tt(pc1, pb, x1c, ALU.subtract)

                # idxp[w(p), r, d', t, corner] = q_i - 1 where pc == d' else -1
                idxp = sp.tile([128, ND, SBM, T, 4], I16, tag="idxp", name="idxp")
                U8 = mybir.dt.uint8
                cmpA = sp.tile(shp, U8, tag="cmpA", name="cmpA")[sl, 0:sn, :]
                cmpB = sp.tile(shp, U8, tag="cmpB", name="cmpB")[sl, 0:sn, :]
                for dpi in range(ND):
                    dp = float(dpi - M)
                    tsc(cmpA, pc0, dp, None, ALU.is_equal)
                    tsc(cmpB, pc1, dp, None, ALU.is_equal)
                    for i in range(4):
                        cmp = cmpA if i in (0, 2) else cmpB
                        nc.vector.select(idxp[sl, dpi, 0:sn, :, i],
                                         cmp, q[i], neg1)

                return (w4p, idxp)

            def emit_R(k, w4p, idxp):
                """repartition candidates to source-column partitions."""
                sn = SBS[k]
                data_sb = sp.tile([128, SBM, KY, ND, TL, 4], BF16,
                                  tag="data_sb", name="data_sb")
                idx_sb = sp.tile([128, SBM, KY, ND, TL, 4], I16,
                                 tag="idx_sb", name="idx_sb")
                nc.vector.memset(idx_sb[:, :, :, :, :, :], -1)
                for dpi in range(ND):
                    dp = dpi - M
                    w_lo = max(0, -dp)
                    w_hi = min(W, W - dp)
                    if w_hi <= w_lo:
                        continue
                    s_lo = w_lo + dp
                    npart = w_hi - w_lo
                    wsrc = w4p[s_lo:s_lo + npart, 0:sn, :, :]
                    nc.sync.dma_start(
                        data_sb[w_lo:w_hi, 0:sn, :, dpi, :, :], wsrc)
                    isrc = idxp[s_lo:s_lo + npart, dpi, 0:sn, :, :]
                    nc.sync.dma_start(
                        idx_sb[w_lo:w_hi, 0:sn, :, dpi, :, :], isrc)
                return data_sb, idx_sb

            def emit_E(k, data_sb, idx_sb, fillers=()):
                """scatter (Pool) + sampling matmuls (PE) + val copies (ACT);
                main GEMM + BN partials + y->DRAM every RB rows."""
                s0 = sum(SBS[:k])
                sn = SBS[k]
                fillers = list(fillers)
                for r in range(sn):
                    if fillers:
                        fillers.pop(0)()
                    j = s0 + r
                    g = j + WIN + 1
                    if j % 2 == 0 and g < cfg.PMROWS:
                        load_win(g, 2 if g + 1 < cfg.PMROWS else 1)
                    s_sb = ssp.tile([128, KY, GW, TL, W], BF16, tag="s_sb")
                    for gky in range(KY):
                        nc.gpsimd.local_scatter(
                            out_ap=s_sb[:, gky, :, :, :].rearrange(
                                "p a b c -> p (a b c)"),
                            data_ap=data_sb[:, r, gky, :, :, :].rearrange(
                                "p a b c -> p (a b c)"),
                            idxs_ap=idx_sb[:, r, gky, :, :, :].rearrange(
                                "p a b c -> p (a b c)"),
                            channels=128,
                            num_elems=cfg.GSLOT,
                            num_idxs=cfg.NCAND,
                        )
                    pv = {}
                    for gky in range(KY):
                        for ci in range(CC):
                            pv[(gky, ci)] = ps_a.tile([128, TL * W], F32,
                                                      tag="sel", name="pv")
                    for dr in range(WIN):
                        wrow = win[:, (j + dr) % WINB, :]
                        for ci in range(CC):
                            for gky in range(KY):
                                drl = dr - gky
                                if not (0 <= drl < GW):
                                    continue
                                nc.tensor.matmul(
                                    pv[(gky, ci)][:, :],
                                    lhsT=wrow[:, ci * 128:(ci + 1) * 128],
                                    rhs=s_sb[:, gky, drl, :, :].rearrange(
                                        "p a b -> p (a b)"),
                                    start=(drl == 0), stop=(drl == GW - 1))
                    for gky in range(KY):
                        for ci in range(CC):
                            nc.scalar.activation(
                                val_sb[:, ci, j % RB, gky * GSL:(gky + 1) * GSL],
                                pv[(gky, ci)][:, :], ACTF.Copy)

                    if (j + 1) % RB == 0 or j == NR - 1:
                        rbeg = (j // RB) * RB
                        rcnt = j - rbeg + 1
                        fidx = j // RB
                        valv = val_sb[:, :, :, :].rearrange(
                            "p c r (t w) -> p c r t w", t=T)
                        yv = y_scr[:, :].rearrange(
                            "p (c r w) -> p c r w", c=OC, w=W)
                        for oi in range(OC):
                            pys = [ps_b.tile([128, RB * wsp], F32, tag="mm",
                                             name="py")
                                   for _ in range(nsp)]
                            outs = [pys[s][:, 0:rcnt * wsp].rearrange(
                                "p (r w) -> p r w", w=wsp) for s in range(nsp)]
                            for t in range(T):
                                for ci in range(CC):
                                    for s in range(nsp):
                                        nc.tensor.matmul(
                                            outs[s],
                                            lhsT=wct[:, t * CC + ci,
                                                     oi * 128:(oi + 1) * 128],
                                            rhs=valv[:, ci, 0:rcnt, t,
                                                     s * wsp:(s + 1) * wsp],
                                            start=(t == 0 and ci == 0),
                                            stop=(t == T - 1 and ci == CC - 1))
                            ystage = wp.tile([128, nsp, RB, wsp], BF16, tag="yst")
                            for s in range(nsp):
                                scrap = wp.tile([128, RB * wsp], F32, tag="scr")
                                pslot = fidx * nsp + s
                                nc.scalar.activation(
                                    ystage[:, s, 0:rcnt, :].rearrange(
                                        "p r w -> p (r w)"),
                                    pys[s][:, 0:rcnt * wsp],
                                    ACTF.Copy,
                                    accum_out=parts[:, 0, oi, pslot:pslot + 1])
                                nc.scalar.activation(
                                    scrap[:, 0:rcnt * wsp], pys[s][:, 0:rcnt * wsp],
                                    ACTF.Square,
                                    accum_out=parts[:, 1, oi, pslot:pslot + 1])
                            for s in range(nsp):
                                nc.sync.dma_start(
                                    yv[:, oi, rbeg:rbeg + rcnt,
                                       s * wsp:(s + 1) * wsp],
                                    ystage[:, s, 0:rcnt, :])

            # ---------------- pipelined emission ----------------
            om0 = emit_B(0)
            emit_C(0, om0)
            w4p0, idxp0 = emit_D(0)
            dsb = emit_R(0, w4p0, idxp0)
            nc.sync.dma_start(xc[:, :, XCHUNK:NR + 2, :],
                              xcmv[:, :, XCHUNK:NR + 2, :])
            nc.sync.dma_start(wct[:, :, :],
                              w_ct_t[:, :].rearrange("p (k o) -> p k o", k=T * CC))
            nc.sync.dma_start(gb[:, :], gb_t[:, :])
            om1 = emit_B(1)
            emit_C(1, om1)
            for k in range(len(SBS)):
                holder = {}
                fillers = []
                if k + 1 < len(SBS):
                    fillers += dr_filler(k + 1, holder)
                if k + 2 < len(SBS):
                    fillers += bc_fillers(k + 2)
                emit_E(k, *dsb, fillers=fillers)
                dsb = holder.get("dsb")

            ssp_cm.__exit__(None, None, None)
            wp_cm.__exit__(None, None, None)
            sp_cm.__exit__(None, None, None)
            tp_cm = tc.tile_pool(name="tailp", bufs=2)
            tp = tp_cm.__enter__()

            # prefetch y readback while stats/collective run
            ysv = y_scr[:, :].rearrange("p (c n) -> p c n", c=OC)
            ytmps = []
            for oi in range(OC):
                ytmp = tp.tile([128, NP], BF16, tag="ytmp")
                nc.sync.dma_start(ytmp[:, :], ysv[:, oi, :])
                ytmps.append(ytmp)

            # ---------------- BN stats finish (+ allreduce) ----------------
            for kind in range(2):
                for oi in range(OC):
                    nc.vector.tensor_reduce(
                        stats[:, kind * OC + oi:kind * OC + oi + 1],
                        parts[:, kind, oi, :], axis=AX.X, op=ALU.add)
            if cfg.use_collective:
                nc.sync.dma_start(cc_in[:, :], stats[:, :])
                nc.gpsimd.collective_compute(
                    "AllReduce", ALU.add,
                    replica_groups=[list(range(cfg.n_cores))],
                    ins=[cc_in[:, :]], outs=[cc_out[:, :]])
                nc.sync.dma_start(stats[:, :], cc_out[:, :])

            # ---------------- affine + SiLU ----------------
            cnt = cfg.bn_count
            mean = cp.tile([128, OC], F32)
            var = cp.tile([128, OC], F32)
            aa = cp.tile([128, OC], F32)
            bb = cp.tile([128, OC], F32)
            sq1 = cp.tile([128, OC], F32)
            nc.vector.tensor_scalar(mean[:, :], stats[:, 0:OC],
                                    1.0 / cnt, None, op0=ALU.mult)
            nc.vector.tensor_scalar(var[:, :], stats[:, OC:2 * OC],
                                    1.0 / cnt, None, op0=ALU.mult)
            nc.vector.tensor_tensor(sq1[:, :], mean[:, :], mean[:, :],
                                    op=ALU.mult)
            nc.vector.tensor_tensor(var[:, :], var[:, :], sq1[:, :],
                                    op=ALU.subtract)
            nc.vector.tensor_scalar(var[:, :], var[:, :], cfg.eps,
                                    None, op0=ALU.add)
            nc.scalar.sqrt(var[:, :], var[:, :])
            nc.vector.reciprocal(var[:, :], var[:, :])
            nc.vector.tensor_tensor(aa[:, :], gb[:, 0:OC], var[:, :],
                                    op=ALU.mult)
            nc.vector.tensor_tensor(bb[:, :], mean[:, :], aa[:, :],
                                    op=ALU.mult)
            nc.vector.tensor_tensor(bb[:, :], gb[:, OC:2 * OC], bb[:, :],
                                    op=ALU.subtract)
            yov = y_out[:, :].rearrange("p (c n) -> p c n", c=OC)
            NPH = NP // 2
            for oi in range(OC):
                ytmp = ytmps[oi]
                for h in range(2):
                    hs = slice(h * NPH, (h + 1) * NPH)
                    out_t = tp.tile([128, NPH], F32, tag="out_t", name="out_t")
                    nc.scalar.activation(out_t[:, :], ytmp[:, hs], ACTF.Silu,
                                         bias=bb[:, oi:oi + 1],
                                         scale=aa[:, oi:oi + 1])
                    nc.sync.dma_start(yov[:, oi, hs], out_t[:, :])
            tp_cm.__exit__(None, None, None)

    nc.compile()
    return nc


# ======================= host side =======================

def host_prepare(inputs: dict, cfg: Cfg = CFG):
    x = np.asarray(inputs["x"], np.float32)
    w_om = np.asarray(inputs["w_om"], np.float32)
    b_om = np.asarray(inputs["b_om"], np.float32)
    weight = np.asarray(inputs["weight"], np.float32)
    gamma = np.asarray(inputs["gamma"], np.float32)
    beta = np.asarray(inputs["beta"], np.float32)
    # conv bias cancels inside batch-stat BN (shift-invariant) — not needed.
    C, CO, H, W, NR, T, M = cfg.C, cfg.CO, cfg.H, cfg.W, cfg.NR, cfg.T, cfg.M
    CC, OC, TL = cfg.CC, cfg.OC, cfg.TL
    B = x.shape[0]
    halves = max(cfg.n_cores // B, 1)
    XW = W + 2

    w_om_l = np.zeros((128, T, CC, 32), BF16_NP)
    for t in range(T):
        ky, kx = t // 3, t % 3
        for ci in range(CC):
            w_om_l[:, t, ci, :27] = w_om[:, ci * 128:(ci + 1) * 128, ky, kx].T.astype(BF16_NP)
    b_om_t = np.zeros((32, 1), np.float32)
    b_om_t[:27, 0] = b_om
    w_ct = np.zeros((128, T * CC, CO), BF16_NP)
    for t in range(T):
        ky, kx = t // 3, t % 3
        for ci in range(CC):
            w_ct[:, t * CC + ci, :] = weight[:, ci * 128:(ci + 1) * 128, ky, kx].T.astype(BF16_NP)
    gb = np.zeros((128, 2 * OC), np.float32)
    for oi in range(OC):
        gb[:, oi] = gamma[oi * 128:(oi + 1) * 128]
        gb[:, OC + oi] = beta[oi * 128:(oi + 1) * 128]

    jj, tt_ = np.meshgrid(np.arange(NR), np.arange(T), indexing="ij")
    ky_m = (tt_ // 3).astype(np.float32)
    kx_m = (tt_ % 3).astype(np.float32)
    tloc_m = (tt_ % 3).astype(np.float32)
    wcol = np.arange(W, dtype=np.float32).reshape(W, 1, 1)

    in_maps = []
    for core in range(cfg.n_cores):
        b = core // halves
        r0 = (core % halves) * NR
        xcm = np.zeros((128, CC, NR + 2, XW), BF16_NP)
        lo = r0 - 1
        src_lo, src_hi = max(lo, 0), min(r0 + NR + 1, H)
        xs_ = x[b, :, src_lo:src_hi, :]
        for ci in range(CC):
            xcm[:, ci, (src_lo - lo):(src_lo - lo) + xs_.shape[1], 1:1 + W] = \
                xs_[ci * 128:(ci + 1) * 128].astype(BF16_NP)
        xcm = xcm.reshape(128, CC * (NR + 2) * XW)

        # pixel-major x, rows padded to 128 columns, M pad rows top/bottom
        xp = np.zeros((cfg.PMROWS, 128, C), BF16_NP)
        gl_lo = r0 - M
        g0, g1 = max(gl_lo, 0), min(gl_lo + cfg.PMROWS, H)
        xp[g0 - gl_lo:g1 - gl_lo, :W, :] = np.transpose(
            x[b, :, g0:g1, :], (1, 2, 0)).astype(BF16_NP)
        x_pm = xp.reshape(cfg.PMROWS * 128, C)

        rj = (r0 + jj).astype(np.float32)
        ybase = np.broadcast_to((rj + ky_m - 1)[None], (W, NR, T))
        xbase = wcol + np.broadcast_to((kx_m - 1)[None], (W, NR, T))
        rjmap = np.broadcast_to(rj[None], (W, NR, T))
        ibconst = (-ky_m * (TL * W) + tloc_m * W)[None] + wcol
        pb = np.broadcast_to(wcol, (W, NR, T))
        coef = np.stack([np.ascontiguousarray(a, np.float32) for a in
                         (ybase, xbase, rjmap, ibconst, pb)], axis=1)
        in_maps.append(dict(
            x_cm=xcm,
            x_pm=x_pm,
            w_om_l=w_om_l.reshape(128, T * CC * 32),
            b_om_t=b_om_t,
            w_ct_t=w_ct.reshape(128, T * CC * CO),
            coef_t=coef.reshape(W, 5 * NR * T),
            gb_t=gb,
        ))
    return in_maps


def reassemble(results, cfg: Cfg = CFG):
    B, halves = cfg.B_total, max(cfg.n_cores // cfg.B_total, 1)
    H, W, NR = cfg.H, cfg.W, cfg.NR
    y = np.zeros((B, cfg.CO, H, W), np.float32)
    for core, res in enumerate(results):
        b = core // halves
        r0 = (core % halves) * NR
        yo = np.asarray(res["y_out"]).reshape(128, cfg.OC, NR, W)
        for oi in range(cfg.OC):
            y[b, oi * 128:(oi + 1) * 128, r0:r0 + NR, :] = yo[:, oi]
    return y


_NC_CACHE = {}


def kernel(**inputs) -> np.ndarray:
    from concourse.bass_utils import run_bass_kernel_spmd
    cfg = CFG
    if "nc" not in _NC_CACHE:
        _NC_CACHE["nc"] = build_nc(cfg)
    nc = _NC_CACHE["nc"]
    in_maps = host_prepare(inputs, cfg)
    res = run_bass_kernel_spmd(nc, in_maps, core_ids=list(range(cfg.n_cores)))
    return reassemble(res.results, cfg)
